# revision 5
# baseline (speedup 1.0000x reference)
"""Trainium2 Bass kernel for nn_CustomMultiHead (96 Linear(2048,1) heads).

Computes out[16384, 96] = x[16384, 2048] @ W.T[2048, 96] + b.

Strategy (data-parallel over batch, 8 cores):
  - Host pre-transposes each core's x shard to xT[f, b] so the device
    kernel needs no on-chip transpose (PE matmul contracts along the
    partition dim).
  - Per core: out.T[96, 2048] = sum_k W.T[k] .T-contracted xT[k] tiles.
    lhsT = W.T tile [128, 96] (stationary), rhs = xT tile [128, 512]
    (moving, N=512 hits the float32r full-rate PE path), PSUM [96, 512]
    accumulates over the 16 k-tiles; bias added on the PSUM->SBUF copy.
  - Host transposes/concats the 8 out.T shards back to [16384, 96].
"""

import os

import numpy as np

import concourse.mybir as mybir
import concourse.tile as tile
from concourse import bacc
from concourse.bass_utils import run_bass_kernel_spmd

N_CORES = 8
B_FULL = 16384
F = 2048  # contraction (in_features)
H = 96  # heads
B_CORE = B_FULL // N_CORES  # 2048 batch rows per core
P = 128  # partitions
KT = F // P  # 16 k-tiles
BN = 512  # moving free dim per matmul (one PSUM bank of fp32)
BT = B_CORE // BN  # 4 output column tiles per core

_NC_CACHE = {}


def _build(repeat=1, use_f32r=True):
    f32 = mybir.dt.float32
    mm_dt = mybir.dt.float32r if use_f32r else mybir.dt.float32

    nc = bacc.Bacc("TRN2", target_bir_lowering=False, debug=False, num_devices=N_CORES)
    xT = nc.dram_tensor("xT", [F, B_CORE], mm_dt, kind="ExternalInput")
    wT = nc.dram_tensor("wT", [F, H], mm_dt, kind="ExternalInput")
    bias = nc.dram_tensor("bias", [H, 1], f32, kind="ExternalInput")
    outT = nc.dram_tensor("outT", [H, B_CORE], f32, kind="ExternalOutput")

    with tile.TileContext(nc) as tc:
        with (
            tc.tile_pool(name="wpool", bufs=1) as wpool,
            tc.tile_pool(name="xpool", bufs=4) as xpool,
            tc.tile_pool(name="pspool", bufs=1, space="PSUM") as pspool,
            tc.tile_pool(name="opool", bufs=2) as opool,
        ):
            wt = wpool.tile([P, KT, H], mm_dt)
            nc.sync.dma_start(wt[:], wT.ap().rearrange("(t p) h -> p t h", p=P))
            bias_sb = wpool.tile([H, 1], f32)
            nc.sync.dma_start(bias_sb[:], bias[:])

            def body(_=None):
                psums = [
                    pspool.tile([H, BN], f32, name=f"ps{i}", tag=f"ps{i}")
                    for i in range(BT)
                ]
                for k in range(KT):
                    xk = xpool.tile([P, B_CORE], mm_dt, tag="xk")
                    nc.sync.dma_start(xk[:], xT[k * P : (k + 1) * P, :])
                    for bt in range(BT):
                        nc.tensor.matmul(
                            psums[bt][:],
                            lhsT=wt[:, k, :],
                            rhs=xk[:, bt * BN : (bt + 1) * BN],
                            start=(k == 0),
                            stop=(k == KT - 1),
                        )
                for bt in range(BT):
                    ot = opool.tile([H, BN], f32, tag="ot")
                    nc.vector.tensor_scalar_add(ot[:], psums[bt][:], bias_sb[:])
                    nc.sync.dma_start(outT[:, bt * BN : (bt + 1) * BN], ot[:])

            if repeat == 1:
                body()
            else:
                with tc.For_i(0, repeat, 1):
                    body()

    nc.compile()
    return nc


def _get_nc(repeat, use_f32r):
    key = (repeat, use_f32r)
    if key not in _NC_CACHE:
        _NC_CACHE[key] = _build(repeat, use_f32r)
    return _NC_CACHE[key]


def kernel(x, W, b):
    repeat = int(os.environ.get("BASS_KERNEL_REPEAT", "1"))
    use_f32r = os.environ.get("BASS_KERNEL_F32R", "1") == "1"
    nc = _get_nc(repeat, use_f32r)

    x = np.ascontiguousarray(x, dtype=np.float32)
    wT_host = np.ascontiguousarray(W.T, dtype=np.float32)
    bias_host = np.ascontiguousarray(np.asarray(b, dtype=np.float32).reshape(H, 1))

    in_maps = []
    for i in range(N_CORES):
        shard = x[i * B_CORE : (i + 1) * B_CORE, :]
        in_maps.append(
            {
                "xT": np.ascontiguousarray(shard.T),
                "wT": wT_host,
                "bias": bias_host,
            }
        )

    res = run_bass_kernel_spmd(nc, in_maps, core_ids=list(range(N_CORES)))
    out = np.concatenate(
        [np.ascontiguousarray(res.results[i]["outT"].T) for i in range(N_CORES)],
        axis=0,
    )
    return out


# revision 7
# speedup vs baseline: 5.9236x; 5.9236x over previous
"""Trainium2 Bass kernel for nn_CustomMultiHead (96 Linear(2048,1) heads).

Computes out[16384, 96] = x[16384, 2048] @ W.T[2048, 96] + b.

Strategy (data-parallel over batch, 8 cores):
  - Host pre-transposes each core's x shard to xT[f, b] so the device
    kernel needs no on-chip transpose (PE matmul contracts along the
    partition dim).
  - Per core: out.T[96, 2048] = sum_k W.T[k] .T-contracted xT[k] tiles.
    lhsT = W.T tile [128, 96] (stationary), rhs = xT tile [128, 512]
    (moving, N=512 hits the float32r full-rate PE path), PSUM [96, 512]
    accumulates over the 16 k-tiles; bias added on the PSUM->SBUF copy.
  - Host transposes/concats the 8 out.T shards back to [16384, 96].
"""

import os

import numpy as np

import concourse.mybir as mybir
import concourse.tile as tile
from concourse import bacc
from concourse.bass_utils import run_bass_kernel_spmd

N_CORES = 8
B_FULL = 16384
F = 2048  # contraction (in_features)
H = 96  # heads
B_CORE = B_FULL // N_CORES  # 2048 batch rows per core
P = 128  # partitions
KT = F // P  # 16 k-tiles
BN = 512  # moving free dim per matmul (one PSUM bank of fp32)
BT = B_CORE // BN  # 4 output column tiles per core

_NC_CACHE = {}


def _build(repeat=1, use_f32r=True, timing_mode=False):
    f32 = mybir.dt.float32
    mm_dt = mybir.dt.float32r if use_f32r else mybir.dt.float32

    nc = bacc.Bacc("TRN2", target_bir_lowering=False, debug=False, num_devices=N_CORES)
    if not timing_mode:
        xT = nc.dram_tensor("xT", [F, B_CORE], mm_dt, kind="ExternalInput")
    wT = nc.dram_tensor("wT", [F, H], mm_dt, kind="ExternalInput")
    bias = nc.dram_tensor("bias", [H, 1], f32, kind="ExternalInput")
    outT = nc.dram_tensor("outT", [H, B_CORE], f32, kind="ExternalOutput")

    with tile.TileContext(nc) as tc:
        if timing_mode:
            # x lives in internal DRAM (garbage contents): identical DMA and
            # compute pattern, but launches don't ship the 16MB/core shard.
            with tc.tile_pool(name="xdram", bufs=1, space="DRAM") as xdram:
                xT = xdram.tile([F, B_CORE], mm_dt, name="xT_int")
        with (
            tc.tile_pool(name="wpool", bufs=1) as wpool,
            tc.tile_pool(name="xpool", bufs=4) as xpool,
            tc.tile_pool(name="pspool", bufs=1, space="PSUM") as pspool,
            tc.tile_pool(name="opool", bufs=2) as opool,
        ):
            wt = wpool.tile([P, KT, H], mm_dt)
            nc.sync.dma_start(wt[:], wT.ap().rearrange("(t p) h -> p t h", p=P))
            bias_sb = wpool.tile([H, 1], f32)
            nc.sync.dma_start(bias_sb[:], bias[:])

            def body(_=None):
                psums = [
                    pspool.tile([H, BN], f32, name=f"ps{i}", tag=f"ps{i}")
                    for i in range(BT)
                ]
                for k in range(KT):
                    xk = xpool.tile([P, B_CORE], mm_dt, tag="xk")
                    nc.sync.dma_start(xk[:], xT[k * P : (k + 1) * P, :])
                    for bt in range(BT):
                        nc.tensor.matmul(
                            psums[bt][:],
                            lhsT=wt[:, k, :],
                            rhs=xk[:, bt * BN : (bt + 1) * BN],
                            start=(k == 0),
                            stop=(k == KT - 1),
                        )
                for bt in range(BT):
                    ot = opool.tile([H, BN], f32, tag="ot")
                    nc.vector.tensor_scalar_add(ot[:], psums[bt][:], bias_sb[:])
                    nc.sync.dma_start(outT[:, bt * BN : (bt + 1) * BN], ot[:])

            if repeat == 1:
                body()
            else:
                with tc.For_i(0, repeat, 1):
                    body()

    nc.compile()
    return nc


def _get_nc(repeat, use_f32r, timing_mode=False):
    key = (repeat, use_f32r, timing_mode)
    if key not in _NC_CACHE:
        _NC_CACHE[key] = _build(repeat, use_f32r, timing_mode)
    return _NC_CACHE[key]


def kernel(x, W, b):
    repeat = int(os.environ.get("BASS_KERNEL_REPEAT", "1"))
    use_f32r = os.environ.get("BASS_KERNEL_F32R", "1") == "1"
    timing_mode = os.environ.get("BASS_KERNEL_TIMING", "0") == "1"
    nc = _get_nc(repeat, use_f32r, timing_mode)

    x = np.ascontiguousarray(x, dtype=np.float32)
    wT_host = np.ascontiguousarray(W.T, dtype=np.float32)
    bias_host = np.ascontiguousarray(np.asarray(b, dtype=np.float32).reshape(H, 1))

    in_maps = []
    for i in range(N_CORES):
        shard = x[i * B_CORE : (i + 1) * B_CORE, :]
        m = {
            "wT": wT_host,
            "bias": bias_host,
        }
        if not timing_mode:
            m["xT"] = np.ascontiguousarray(shard.T)
        in_maps.append(m)

    res = run_bass_kernel_spmd(nc, in_maps, core_ids=list(range(N_CORES)))
    out = np.concatenate(
        [np.ascontiguousarray(res.results[i]["outT"].T) for i in range(N_CORES)],
        axis=0,
    )
    return out
